# revision 2
# baseline (speedup 1.0000x reference)
"""ALiBi multi-head causal attention on 8 TRN2 NeuronCores (Bass/Tile SPMD).

Problem: B=4, S=2048, D=1024, H=16, Kd=64 (nn_AlibiMultiHeadAttention).

Sharding (identical SPMD graph on every core):
  core c -> batch b = c//2, head-group g = c%2 (heads g*8 .. g*8+7), ALL 2048 rows.
  Each core: QKV projections for its 8 heads, causal+ALiBi attention (transposed-
  logits layout, ALiBi folded into the exp() bias, causal via block skipping +
  one diagonal mask template), then a 2-rank AllGather within each batch pair to
  assemble all 16 heads' attention output, and the output projection for the
  core's half of the rows (row half = g). Host concatenates the 8 [1024,1024]
  row-slices into the full [4,2048,1024] output.

Math note: softmax is shift-invariant per row, so the ALiBi term slope*(i-j)
reduces to the key-side column term -slope*j, which is constant per PSUM
partition in the transposed-logits layout -> it rides the ACT bias operand of
exp() for free. exp() is computed without max-subtraction (logits are O(6)
for unit-normal inputs; fp32 exp overflows only at 88).
"""
import math
import os
import sys

sys.path.insert(0, "/opt/trn_rl_repo")

import numpy as np
import ml_dtypes

import concourse.bass as bass
import concourse.tile as tile
from concourse import bacc, mybir
from concourse.bass_utils import run_bass_kernel_spmd

F32 = mybir.dt.float32
BF16 = mybir.dt.bfloat16
AF = mybir.ActivationFunctionType
ALU = mybir.AluOpType

B, S, D = 4, 2048, 1024
H, KD = 16, 64
HPG = 8                 # heads per core (head-group)
PG = HPG * KD           # 512: p-slice per head-group
NT = S // 128           # 16 token tiles
ND = D // 128           # 8 d tiles
NQC = 4                 # q-chunks of 512
QC = 512
VW = KD + 1             # v' width per head (64 + ones col)
SCALE = 1.0 / math.sqrt(KD)
NEG = -1.0e9
N_CORES = 8
GROUPS = [[0, 1], [2, 3], [4, 5], [6, 7]]
NPT_G = PG // 128

_NC = None
LAST_EXEC_NS = None


def _slopes():
    start = 2.0 ** (-(2.0 ** -(math.log2(H) - 3)))
    return np.array([start * start ** i for i in range(H)], dtype=np.float32)


def _build():
    nc = bacc.Bacc("TRN2", target_bir_lowering=False, num_devices=N_CORES)
    xq = nc.declare_dram_parameter("xq", [S, D], F32, isOutput=False)
    xk = nc.declare_dram_parameter("xk", [S, D], F32, isOutput=False)
    xv = nc.declare_dram_parameter("xv", [S, D], F32, isOutput=False)
    Wq = nc.declare_dram_parameter("Wq", [D, PG], F32, isOutput=False)
    Wk = nc.declare_dram_parameter("Wk", [D, PG], F32, isOutput=False)
    Wv = nc.declare_dram_parameter("Wv", [D, PG], F32, isOutput=False)
    Wo = nc.declare_dram_parameter("Wo", [D, D], F32, isOutput=False)
    bqr = nc.declare_dram_parameter("bqr", [128, PG // 128], F32, isOutput=False)
    bkr = nc.declare_dram_parameter("bkr", [128, PG // 128], F32, isOutput=False)
    bvr = nc.declare_dram_parameter("bvr", [128, PG], F32, isOutput=False)
    bor = nc.declare_dram_parameter("bor", [128, D], F32, isOutput=False)
    alibi = nc.declare_dram_parameter("alibi", [128, HPG * NT], F32, isOutput=False)
    maskT = nc.declare_dram_parameter("maskT", [128, 128], F32, isOutput=False)
    ident = nc.declare_dram_parameter("ident", [128, 128], BF16, isOutput=False)
    out = nc.declare_dram_parameter("out", [S // 2, D], F32, isOutput=True)

    NPT = PG // 128  # 4 p-tiles per head-group (2 heads each)

    with tile.TileContext(nc) as tc:
        with (
            tc.tile_pool(name="const", bufs=1) as constp,
            tc.tile_pool(name="wo", bufs=1) as wop,
            tc.tile_pool(name="proj_out", bufs=1) as projp,
            tc.tile_pool(name="attn_sb", bufs=1) as attnp,
            tc.tile_pool(name="dram", bufs=1, space="DRAM") as dramp,
        ):
            maskT_t = constp.tile([128, 128], F32)
            nc.sync.dma_start(maskT_t[:], maskT[:])
            alibi_t = constp.tile([128, HPG * NT], F32)
            nc.sync.dma_start(alibi_t[:], alibi[:])
            bqr_t = constp.tile([128, NPT], F32)
            nc.sync.dma_start(bqr_t[:], bqr[:])
            bkr_t = constp.tile([128, NPT], F32)
            nc.sync.dma_start(bkr_t[:], bkr[:])
            bvr_t = constp.tile([128, PG], F32)
            nc.sync.dma_start(bvr_t[:], bvr[:])
            bor_t = constp.tile([128, D], F32)
            nc.sync.dma_start(bor_t[:], bor[:])

            Wo_t = [wop.tile([128, D], BF16, name=f"Wo{p}", tag=f"Wo{p}")
                    for p in range(ND)]
            for p in range(ND):
                nc.gpsimd.dma_start(Wo_t[p][:], Wo[p * 128:(p + 1) * 128, :])

            # persistent projection outputs
            qT = [projp.tile([128, S], BF16, name=f"qT{p}", tag=f"qT{p}")
                  for p in range(NPT)]
            kT = [projp.tile([128, S], BF16, name=f"kT{p}", tag=f"kT{p}")
                  for p in range(NPT)]
            vp = [projp.tile([128, HPG * VW], BF16, name=f"vp{t}", tag=f"vp{t}")
                  for t in range(NT)]
            attnT = [attnp.tile([128, S], BF16, name=f"attnT{j}", tag=f"attnT{j}")
                     for j in range(NPT)]

            # ---------------- Phase 1: loads, transposes, projections ---------
            with (
                tc.tile_pool(name="p1_sb", bufs=1) as p1p,
                tc.tile_pool(name="xT", bufs=1) as xtp,
                tc.tile_pool(name="xload", bufs=3) as xlp,
                tc.tile_pool(name="p1_ps", bufs=4, space="PSUM") as ps_tr,
                tc.tile_pool(name="p1_ps2", bufs=3, space="PSUM") as ps_proj,
            ):
                ident_t = p1p.tile([128, 128], BF16)
                nc.sync.dma_start(ident_t[:], ident[:])
                w_tiles = {}
                for name, W in (("Wv", Wv), ("Wk", Wk), ("Wq", Wq)):
                    w_tiles[name] = [
                        p1p.tile([128, PG], BF16, name=f"{name}{j}", tag=f"{name}{j}")
                        for j in range(ND)]
                    for j in range(ND):
                        nc.gpsimd.dma_start(w_tiles[name][j][:],
                                            W[j * 128:(j + 1) * 128, :])

                for xname, x in (("xv", xv), ("xk", xk), ("xq", xq)):
                    xT = [xtp.tile([128, S], BF16, name=f"{xname}T{j}", tag=f"xT{j}")
                          for j in range(ND)]
                    for t in range(NT):
                        xl = xlp.tile([128, D], BF16, tag="xl", name="xl")
                        nc.gpsimd.dma_start(xl[:], x[t * 128:(t + 1) * 128, :])
                        for j in range(ND):
                            ps = ps_tr.tile([128, 128], BF16, tag="tr", name="tr")
                            nc.tensor.transpose(
                                ps[:], xl[:, j * 128:(j + 1) * 128], ident_t[:])
                            nc.vector.tensor_copy(
                                xT[j][:, t * 128:(t + 1) * 128], ps[:])
                    if xname == "xv":
                        for t in range(NT):
                            ps = ps_proj.tile([128, PG], F32, tag="pj", name="pj")
                            for j in range(ND):
                                nc.tensor.matmul(
                                    ps[:], xT[j][:, t * 128:(t + 1) * 128],
                                    w_tiles["Wv"][j][:],
                                    start=(j == 0), stop=(j == ND - 1))
                            dst = vp[t][:].rearrange("a (h w) -> a h w", w=VW)[:, :, 0:KD]
                            nc.vector.tensor_tensor(
                                dst, ps[:].rearrange("a (h k) -> a h k", k=KD),
                                bvr_t[:].rearrange("a (h k) -> a h k", k=KD),
                                ALU.add)
                            ones_col = vp[t][:].rearrange(
                                "a (h w) -> a h w", w=VW)[:, :, KD:VW]
                            nc.gpsimd.memset(ones_col, 1.0)
                    else:
                        dstl, wname, biasr = (
                            (kT, "Wk", bkr_t) if xname == "xk" else (qT, "Wq", bqr_t))
                        for pt in range(NPT):
                            for qc in range(NQC):
                                ps = ps_proj.tile([128, QC], F32, tag="pj", name="pj")
                                for j in range(ND):
                                    nc.tensor.matmul(
                                        ps[:],
                                        w_tiles[wname][j][:, pt * 128:(pt + 1) * 128],
                                        xT[j][:, qc * QC:(qc + 1) * QC],
                                        start=(j == 0), stop=(j == ND - 1))
                                nc.vector.tensor_scalar(
                                    dstl[pt][:, qc * QC:(qc + 1) * QC], ps[:],
                                    biasr[:, pt:pt + 1], None, ALU.add)

            # ---------------- Phase 2: attention + per-pair AllGather ---------
            ag_out = []
            with (
                tc.tile_pool(name="expt", bufs=3) as expp,
                tc.tile_pool(name="attn_tmp", bufs=2) as atmp,
                tc.tile_pool(name="ps_log", bufs=2, space="PSUM") as ps_log,
                tc.tile_pool(name="ps_av", bufs=4, space="PSUM") as ps_av,
            ):
                for h in range(HPG):
                    pj = h // 2
                    off = 64 * (h % 2)
                    kT_h = kT[pj][off:off + 64, :]
                    qT_h = qT[pj][off:off + 64, :]
                    av_ps = [ps_av.tile([VW, QC], F32, tag="av", name="av")
                             for _ in range(NQC)]
                    for st in range(NT):
                        qcs = list(range(st // 4, NQC))
                        for gi in range(0, len(qcs), 2):
                            grp = qcs[gi:gi + 2]
                            ps_l = ps_log.tile([128, 2 * QC], F32, tag="lg", name="lg")
                            for k, qc in enumerate(grp):
                                seg = ps_l[:, k * QC:(k + 1) * QC]
                                nc.tensor.matmul(
                                    seg, kT_h[:, st * 128:(st + 1) * 128],
                                    qT_h[:, qc * QC:(qc + 1) * QC],
                                    start=True, stop=True)
                                if qc == st // 4:
                                    rel = st % 4
                                    if rel > 0:
                                        nc.vector.memset(seg[:, 0:rel * 128], NEG)
                                    nc.vector.tensor_add(
                                        seg[:, rel * 128:(rel + 1) * 128],
                                        seg[:, rel * 128:(rel + 1) * 128],
                                        maskT_t[:])
                            expt = expp.tile([128, 2 * QC], BF16, tag="ex", name="ex")
                            nc.scalar.activation(
                                expt[:, 0:len(grp) * QC], ps_l[:, 0:len(grp) * QC],
                                AF.Exp,
                                bias=alibi_t[:, h * NT + st:h * NT + st + 1],
                                scale=SCALE)
                            for k, qc in enumerate(grp):
                                nc.tensor.matmul(
                                    av_ps[qc][:],
                                    vp[st][:, h * VW:(h + 1) * VW],
                                    expt[:, k * QC:(k + 1) * QC],
                                    start=(st == 0), stop=(st == 4 * qc + 3))
                    for qc in range(NQC):
                        recip = atmp.tile([1, QC], F32, tag="rc", name="rc")
                        nc.vector.reciprocal(recip[:], av_ps[qc][KD:VW, :])
                        recipb = atmp.tile([KD, QC], F32, tag="rb", name="rb")
                        nc.gpsimd.partition_broadcast(recipb[:], recip[:])
                        nc.vector.tensor_tensor(
                            attnT[pj][off:off + 64, qc * QC:(qc + 1) * QC],
                            av_ps[qc][0:KD, :], recipb[:], ALU.mult)
                    if h % 2 == 1:
                        agi = dramp.tile([128, S], BF16, name=f"agi{pj}", tag=f"agi{pj}")
                        ago = dramp.tile([256, S], BF16, name=f"ago{pj}", tag=f"ago{pj}")
                        nc.sync.dma_start(agi[:], attnT[pj][:])
                        nc.gpsimd.collective_compute(
                            "AllGather", ALU.bypass, replica_groups=GROUPS,
                            ins=[agi.opt()], outs=[ago.opt()])
                        ag_out.append(ago)

            # ---------------- Phase 3: output projection ----------------------
            with (
                tc.tile_pool(name="attnF", bufs=1) as afp,
                tc.tile_pool(name="osb", bufs=2) as osbp,
                tc.tile_pool(name="ps_out", bufs=3, space="PSUM") as ps_out,
            ):
                pid = nc.gpsimd.partition_id()
                qoff = (pid & 1) * (S // 2)
                attnF = [afp.tile([128, S // 2], BF16, name=f"aF{p}", tag=f"aF{p}")
                         for p in range(ND)]
                for grp_i in range(2):       # head-group slot in AG output
                    for pj in range(NPT):
                        nc.gpsimd.dma_start(
                            attnF[grp_i * NPT + pj][:],
                            ag_out[pj][grp_i * 128:(grp_i + 1) * 128,
                                       bass.ds(qoff, S // 2)])
                for lt in range(ND):
                    osb = osbp.tile([128, D], F32, tag="osb", name="osb")
                    for dc in range(2):
                        ps = ps_out.tile([128, QC], F32, tag="po", name="po")
                        for p in range(ND):
                            nc.tensor.matmul(
                                ps[:], attnF[p][:, lt * 128:(lt + 1) * 128],
                                Wo_t[p][:, dc * QC:(dc + 1) * QC],
                                start=(p == 0), stop=(p == ND - 1))
                        nc.vector.tensor_tensor(
                            osb[:, dc * QC:(dc + 1) * QC], ps[:],
                            bor_t[:, dc * QC:(dc + 1) * QC], ALU.add)
                    nc.sync.dma_start(out[lt * 128:(lt + 1) * 128, :], osb[:])

    nc.finalize()
    return nc


def _get_nc():
    global _NC
    if _NC is None:
        _NC = _build()
    return _NC


def kernel(query, key, value, Wq, bq, Wk, bk, Wv, bv, Wo, bo):
    global LAST_EXEC_NS
    query = np.asarray(query, np.float32)
    key = np.asarray(key, np.float32)
    value = np.asarray(value, np.float32)
    Wq = np.asarray(Wq, np.float32)
    Wk = np.asarray(Wk, np.float32)
    Wv = np.asarray(Wv, np.float32)
    Wo = np.asarray(Wo, np.float32)
    bq = np.asarray(bq, np.float32)
    bk = np.asarray(bk, np.float32)
    bv = np.asarray(bv, np.float32)
    bo = np.asarray(bo, np.float32)

    slopes = _slopes()
    p_idx = np.arange(128, dtype=np.float32)
    maskT_a = np.where(np.arange(128)[None, :] >= np.arange(128)[:, None],
                       0.0, NEG).astype(np.float32)
    ident_a = np.eye(128).astype(ml_dtypes.bfloat16)
    bor_a = np.ascontiguousarray(np.broadcast_to(bo, (128, D)).astype(np.float32))

    in_maps = []
    for c in range(N_CORES):
        b, g = c // 2, c % 2
        sl = slice(g * PG, (g + 1) * PG)
        alibi_a = np.empty((128, HPG * NT), np.float32)
        for hl in range(HPG):
            sp = slopes[g * HPG + hl]
            for st in range(NT):
                alibi_a[:, hl * NT + st] = -sp * (st * 128 + p_idx)
        in_maps.append(dict(
            xq=np.ascontiguousarray(query[b]),
            xk=np.ascontiguousarray(key[b]),
            xv=np.ascontiguousarray(value[b]),
            Wq=np.ascontiguousarray(Wq[:, sl]),
            Wk=np.ascontiguousarray(Wk[:, sl]),
            Wv=np.ascontiguousarray(Wv[:, sl]),
            Wo=Wo,
            bqr=np.ascontiguousarray(bq[sl].reshape(NPT_G, 128).T),
            bkr=np.ascontiguousarray(bk[sl].reshape(NPT_G, 128).T),
            bvr=np.ascontiguousarray(np.broadcast_to(bv[sl], (128, PG))),
            bor=bor_a,
            alibi=alibi_a,
            maskT=maskT_a,
            ident=ident_a,
        ))

    nc = _get_nc()
    trace = os.environ.get("ALIBI_TRACE", "0") == "1"
    res = run_bass_kernel_spmd(nc, in_maps, core_ids=list(range(N_CORES)),
                               trace=trace)
    if trace:
        LAST_EXEC_NS = res.exec_time_ns

    out_full = np.empty((B, S, D), np.float32)
    for c in range(N_CORES):
        b, g = c // 2, c % 2
        out_full[b, g * (S // 2):(g + 1) * (S // 2), :] = res.results[c]["out"]
    return out_full


NPT_G = PG // 128


# revision 3
# speedup vs baseline: 1.2295x; 1.2295x over previous
"""ALiBi multi-head causal attention on 8 TRN2 NeuronCores (Bass/Tile SPMD).

Problem: B=4, S=2048, D=1024, H=16, Kd=64 (nn_AlibiMultiHeadAttention).

Sharding (identical SPMD graph on every core):
  core c -> batch b = c//2, head-group g = c%2 (heads g*8 .. g*8+7), ALL 2048 rows.
  Each core: QKV projections for its 8 heads, causal+ALiBi attention (transposed-
  logits layout, ALiBi folded into the exp() bias, causal via block skipping +
  one diagonal mask template), then a 2-rank AllGather within each batch pair to
  assemble all 16 heads' attention output, and the output projection for the
  core's half of the rows (row half = g). Host concatenates the 8 [1024,1024]
  row-slices into the full [4,2048,1024] output.

Math note: softmax is shift-invariant per row, so the ALiBi term slope*(i-j)
reduces to the key-side column term -slope*j, which is constant per PSUM
partition in the transposed-logits layout -> it rides the ACT bias operand of
exp() for free. exp() is computed without max-subtraction (logits are O(6)
for unit-normal inputs; fp32 exp overflows only at 88).
"""
import math
import os
import sys

sys.path.insert(0, "/opt/trn_rl_repo")

import numpy as np
import ml_dtypes

import concourse.bass as bass
import concourse.tile as tile
from concourse import bacc, mybir
from concourse.bass_utils import run_bass_kernel_spmd

F32 = mybir.dt.float32
BF16 = mybir.dt.bfloat16
AF = mybir.ActivationFunctionType
ALU = mybir.AluOpType

B, S, D = 4, 2048, 1024
H, KD = 16, 64
HPG = 8                 # heads per core (head-group)
PG = HPG * KD           # 512: p-slice per head-group
NT = S // 128           # 16 token tiles
ND = D // 128           # 8 d tiles
NQC = 4                 # q-chunks of 512
QC = 512
VW = 2 * KD             # v' width per head (64 + 64 ones cols -> denom on partitions 64..127)
SCALE = 1.0 / math.sqrt(KD)
NEG = -1.0e9
N_CORES = 8
GROUPS = [[0, 1], [2, 3], [4, 5], [6, 7]]
NPT_G = PG // 128

_NC = None
LAST_EXEC_NS = None


def _slopes():
    start = 2.0 ** (-(2.0 ** -(math.log2(H) - 3)))
    return np.array([start * start ** i for i in range(H)], dtype=np.float32)


def _build():
    nc = bacc.Bacc("TRN2", target_bir_lowering=False, num_devices=N_CORES)
    xq = nc.declare_dram_parameter("xq", [S, D], F32, isOutput=False)
    xk = nc.declare_dram_parameter("xk", [S, D], F32, isOutput=False)
    xv = nc.declare_dram_parameter("xv", [S, D], F32, isOutput=False)
    Wq = nc.declare_dram_parameter("Wq", [D, PG], F32, isOutput=False)
    Wk = nc.declare_dram_parameter("Wk", [D, PG], F32, isOutput=False)
    Wv = nc.declare_dram_parameter("Wv", [D, PG], F32, isOutput=False)
    Wo = nc.declare_dram_parameter("Wo", [D, D], F32, isOutput=False)
    bqr = nc.declare_dram_parameter("bqr", [128, PG // 128], F32, isOutput=False)
    bkr = nc.declare_dram_parameter("bkr", [128, PG // 128], F32, isOutput=False)
    bvr = nc.declare_dram_parameter("bvr", [128, PG], F32, isOutput=False)
    bor = nc.declare_dram_parameter("bor", [128, D], F32, isOutput=False)
    alibi = nc.declare_dram_parameter("alibi", [128, HPG * NT], F32, isOutput=False)
    maskT = nc.declare_dram_parameter("maskT", [128, 128], F32, isOutput=False)
    ident = nc.declare_dram_parameter("ident", [128, 128], BF16, isOutput=False)
    out = nc.declare_dram_parameter("out", [S // 2, D], F32, isOutput=True)

    NPT = PG // 128  # 4 p-tiles per head-group (2 heads each)

    with tile.TileContext(nc) as tc:
        with (
            tc.tile_pool(name="const", bufs=1) as constp,
            tc.tile_pool(name="wo", bufs=1) as wop,
            tc.tile_pool(name="proj_out", bufs=1) as projp,
            tc.tile_pool(name="attn_sb", bufs=1) as attnp,
            tc.tile_pool(name="dram", bufs=1, space="DRAM") as dramp,
        ):
            maskT_t = constp.tile([128, 128], F32)
            nc.sync.dma_start(maskT_t[:], maskT[:])
            alibi_t = constp.tile([128, HPG * NT], F32)
            nc.sync.dma_start(alibi_t[:], alibi[:])
            bqr_t = constp.tile([128, NPT], F32)
            nc.sync.dma_start(bqr_t[:], bqr[:])
            bkr_t = constp.tile([128, NPT], F32)
            nc.sync.dma_start(bkr_t[:], bkr[:])
            bvr_t = constp.tile([128, PG], F32)
            nc.sync.dma_start(bvr_t[:], bvr[:])
            bor_t = constp.tile([128, D], F32)
            nc.sync.dma_start(bor_t[:], bor[:])

            Wo_t = [wop.tile([128, D], BF16, name=f"Wo{p}", tag=f"Wo{p}")
                    for p in range(ND)]
            for p in range(ND):
                nc.gpsimd.dma_start(Wo_t[p][:], Wo[p * 128:(p + 1) * 128, :])

            # persistent projection outputs
            qT = [projp.tile([128, S], BF16, name=f"qT{p}", tag=f"qT{p}")
                  for p in range(NPT)]
            kT = [projp.tile([128, S], BF16, name=f"kT{p}", tag=f"kT{p}")
                  for p in range(NPT)]
            vp = [projp.tile([128, HPG * VW], BF16, name=f"vp{t}", tag=f"vp{t}")
                  for t in range(NT)]
            attnT = [attnp.tile([128, S], BF16, name=f"attnT{j}", tag=f"attnT{j}")
                     for j in range(NPT)]

            # ---------------- Phase 1: loads, transposes, projections ---------
            with (
                tc.tile_pool(name="p1_sb", bufs=1) as p1p,
                tc.tile_pool(name="xT", bufs=2) as xtp,
                tc.tile_pool(name="xload", bufs=3) as xlp,
                tc.tile_pool(name="p1_ps", bufs=4, space="PSUM") as ps_tr,
                tc.tile_pool(name="p1_ps2", bufs=3, space="PSUM") as ps_proj,
            ):
                ident_t = p1p.tile([128, 128], BF16)
                nc.sync.dma_start(ident_t[:], ident[:])
                w_tiles = {}
                for name, W in (("Wv", Wv), ("Wk", Wk), ("Wq", Wq)):
                    w_tiles[name] = [
                        p1p.tile([128, PG], BF16, name=f"{name}{j}", tag=f"{name}{j}")
                        for j in range(ND)]
                    for j in range(ND):
                        nc.gpsimd.dma_start(w_tiles[name][j][:],
                                            W[j * 128:(j + 1) * 128, :])

                for xname, x in (("xv", xv), ("xk", xk), ("xq", xq)):
                    xT = [xtp.tile([128, S], BF16, name=f"{xname}T{j}", tag=f"xT{j}")
                          for j in range(ND)]
                    for t in range(NT):
                        xl = xlp.tile([128, D], BF16, tag="xl", name="xl")
                        nc.gpsimd.dma_start(xl[:], x[t * 128:(t + 1) * 128, :])
                        for j in range(ND):
                            ps = ps_tr.tile([128, 128], BF16, tag="tr", name="tr")
                            nc.tensor.transpose(
                                ps[:], xl[:, j * 128:(j + 1) * 128], ident_t[:])
                            nc.vector.tensor_copy(
                                xT[j][:, t * 128:(t + 1) * 128], ps[:])
                    if xname == "xv":
                        for t in range(NT):
                            ps = ps_proj.tile([128, PG], F32, tag="pj", name="pj")
                            for j in range(ND):
                                nc.tensor.matmul(
                                    ps[:], xT[j][:, t * 128:(t + 1) * 128],
                                    w_tiles["Wv"][j][:],
                                    start=(j == 0), stop=(j == ND - 1))
                            dst = vp[t][:].rearrange("a (h w) -> a h w", w=VW)[:, :, 0:KD]
                            nc.vector.tensor_tensor(
                                dst, ps[:].rearrange("a (h k) -> a h k", k=KD),
                                bvr_t[:].rearrange("a (h k) -> a h k", k=KD),
                                ALU.add)
                            ones_col = vp[t][:].rearrange(
                                "a (h w) -> a h w", w=VW)[:, :, KD:VW]
                            nc.gpsimd.memset(ones_col, 1.0)
                    else:
                        dstl, wname, biasr = (
                            (kT, "Wk", bkr_t) if xname == "xk" else (qT, "Wq", bqr_t))
                        for pt in range(NPT):
                            for qc in range(NQC):
                                ps = ps_proj.tile([128, QC], F32, tag="pj", name="pj")
                                for j in range(ND):
                                    nc.tensor.matmul(
                                        ps[:],
                                        w_tiles[wname][j][:, pt * 128:(pt + 1) * 128],
                                        xT[j][:, qc * QC:(qc + 1) * QC],
                                        start=(j == 0), stop=(j == ND - 1))
                                nc.vector.tensor_scalar(
                                    dstl[pt][:, qc * QC:(qc + 1) * QC], ps[:],
                                    biasr[:, pt:pt + 1], None, ALU.add)

            # ---------------- Phase 2: attention + per-pair AllGather ---------
            ag_out = []
            with (
                tc.tile_pool(name="expt", bufs=3) as expp,
                tc.tile_pool(name="attn_tmp", bufs=2) as atmp,
                tc.tile_pool(name="ps_log", bufs=2, space="PSUM") as ps_log,
                tc.tile_pool(name="ps_av", bufs=4, space="PSUM") as ps_av,
            ):
                pid = nc.gpsimd.partition_id()
                qoff_other = (1 - (pid & 1)) * (S // 2)
                for h in range(HPG):
                    pj = h // 2
                    off = 64 * (h % 2)
                    kT_h = kT[pj][off:off + 64, :]
                    qT_h = qT[pj][off:off + 64, :]
                    av_ps = [ps_av.tile([VW, QC], F32, tag="av", name="av")
                             for _ in range(NQC)]
                    groups = []
                    for st in range(NT):
                        qcs = list(range(st // 4, NQC))
                        for gi in range(0, len(qcs), 2):
                            groups.append((st, qcs[gi:gi + 2]))
                    pend = None   # (st, grp, expt) awaiting AV emission
                    for st, grp in groups:
                        ps_l = ps_log.tile([128, 2 * QC], F32, tag="lg", name="lg")
                        for k, qc in enumerate(grp):
                            seg = ps_l[:, k * QC:(k + 1) * QC]
                            nc.tensor.matmul(
                                seg, kT_h[:, st * 128:(st + 1) * 128],
                                qT_h[:, qc * QC:(qc + 1) * QC],
                                start=True, stop=True)
                            if qc == st // 4:
                                rel = st % 4
                                if rel > 0:
                                    nc.vector.memset(seg[:, 0:rel * 128], NEG)
                                nc.vector.tensor_add(
                                    seg[:, rel * 128:(rel + 1) * 128],
                                    seg[:, rel * 128:(rel + 1) * 128],
                                    maskT_t[:])
                        expt = expp.tile([128, 2 * QC], BF16, tag="ex", name="ex")
                        nc.scalar.activation(
                            expt[:, 0:len(grp) * QC], ps_l[:, 0:len(grp) * QC],
                            AF.Exp,
                            bias=alibi_t[:, h * NT + st:h * NT + st + 1],
                            scale=SCALE)
                        if pend is not None:
                            pst, pgrp, pex = pend
                            for k, qc in enumerate(pgrp):
                                nc.tensor.matmul(
                                    av_ps[qc][:],
                                    vp[pst][:, h * VW:(h + 1) * VW],
                                    pex[:, k * QC:(k + 1) * QC],
                                    start=(pst == 0), stop=(pst == 4 * qc + 3))
                        pend = (st, grp, expt)
                    pst, pgrp, pex = pend
                    for k, qc in enumerate(pgrp):
                        nc.tensor.matmul(
                            av_ps[qc][:], vp[pst][:, h * VW:(h + 1) * VW],
                            pex[:, k * QC:(k + 1) * QC],
                            start=(pst == 0), stop=(pst == 4 * qc + 3))
                    for qc in range(NQC):
                        recipb = atmp.tile([KD, QC], F32, tag="rb", name="rb")
                        nc.vector.reciprocal(recipb[:], av_ps[qc][KD:VW, :])
                        nc.vector.tensor_tensor(
                            attnT[pj][off:off + 64, qc * QC:(qc + 1) * QC],
                            av_ps[qc][0:KD, :], recipb[:], ALU.mult)
                    if h % 2 == 1:
                        agi = dramp.tile([128, S // 2], BF16, name=f"agi{pj}", tag=f"agi{pj}")
                        ago = dramp.tile([256, S // 2], BF16, name=f"ago{pj}", tag=f"ago{pj}")
                        nc.gpsimd.dma_start(
                            agi[:], attnT[pj][:, bass.ds(qoff_other, S // 2)])
                        nc.gpsimd.collective_compute(
                            "AllGather", ALU.bypass, replica_groups=GROUPS,
                            ins=[agi.opt()], outs=[ago.opt()])
                        ag_out.append(ago)

            # ---------------- Phase 3: output projection ----------------------
            with (
                tc.tile_pool(name="attnF", bufs=1) as afp,
                tc.tile_pool(name="osb", bufs=2) as osbp,
                tc.tile_pool(name="ps_out", bufs=3, space="PSUM") as ps_out,
            ):
                pid2 = nc.gpsimd.partition_id()
                qoff_mine = (pid2 & 1) * (S // 2)
                rowoff_partner = (1 - (pid2 & 1)) * 128
                attnF = [afp.tile([128, S // 2], BF16, name=f"aF{p}", tag=f"aF{p}")
                         for p in range(ND)]
                for pj in range(NPT):
                    nc.gpsimd.dma_start(
                        attnF[pj][:], attnT[pj][:, bass.ds(qoff_mine, S // 2)])
                    nc.gpsimd.dma_start(
                        attnF[NPT + pj][:],
                        ag_out[pj][bass.ds(rowoff_partner, 128), :])
                for lt in range(ND):
                    osb = osbp.tile([128, D], F32, tag="osb", name="osb")
                    for dc in range(2):
                        ps = ps_out.tile([128, QC], F32, tag="po", name="po")
                        for p in range(ND):
                            nc.tensor.matmul(
                                ps[:], attnF[p][:, lt * 128:(lt + 1) * 128],
                                Wo_t[p][:, dc * QC:(dc + 1) * QC],
                                start=(p == 0), stop=(p == ND - 1))
                        nc.vector.tensor_tensor(
                            osb[:, dc * QC:(dc + 1) * QC], ps[:],
                            bor_t[:, dc * QC:(dc + 1) * QC], ALU.add)
                    nc.sync.dma_start(out[lt * 128:(lt + 1) * 128, :], osb[:])

    nc.finalize()
    return nc


def _get_nc():
    global _NC
    if _NC is None:
        _NC = _build()
    return _NC


def kernel(query, key, value, Wq, bq, Wk, bk, Wv, bv, Wo, bo):
    global LAST_EXEC_NS
    query = np.asarray(query, np.float32)
    key = np.asarray(key, np.float32)
    value = np.asarray(value, np.float32)
    Wq = np.asarray(Wq, np.float32)
    Wk = np.asarray(Wk, np.float32)
    Wv = np.asarray(Wv, np.float32)
    Wo = np.asarray(Wo, np.float32)
    bq = np.asarray(bq, np.float32)
    bk = np.asarray(bk, np.float32)
    bv = np.asarray(bv, np.float32)
    bo = np.asarray(bo, np.float32)

    slopes = _slopes()
    p_idx = np.arange(128, dtype=np.float32)
    maskT_a = np.where(np.arange(128)[None, :] >= np.arange(128)[:, None],
                       0.0, NEG).astype(np.float32)
    ident_a = np.eye(128).astype(ml_dtypes.bfloat16)
    bor_a = np.ascontiguousarray(np.broadcast_to(bo, (128, D)).astype(np.float32))

    in_maps = []
    for c in range(N_CORES):
        b, g = c // 2, c % 2
        sl = slice(g * PG, (g + 1) * PG)
        alibi_a = np.empty((128, HPG * NT), np.float32)
        for hl in range(HPG):
            sp = slopes[g * HPG + hl]
            for st in range(NT):
                alibi_a[:, hl * NT + st] = -sp * (st * 128 + p_idx)
        in_maps.append(dict(
            xq=np.ascontiguousarray(query[b]),
            xk=np.ascontiguousarray(key[b]),
            xv=np.ascontiguousarray(value[b]),
            Wq=np.ascontiguousarray(Wq[:, sl]),
            Wk=np.ascontiguousarray(Wk[:, sl]),
            Wv=np.ascontiguousarray(Wv[:, sl]),
            Wo=np.ascontiguousarray(
                np.concatenate([Wo[g * PG:(g + 1) * PG, :],
                                Wo[(1 - g) * PG:(2 - g) * PG, :]], axis=0)),
            bqr=np.ascontiguousarray(bq[sl].reshape(NPT_G, 128).T),
            bkr=np.ascontiguousarray(bk[sl].reshape(NPT_G, 128).T),
            bvr=np.ascontiguousarray(np.broadcast_to(bv[sl], (128, PG))),
            bor=bor_a,
            alibi=alibi_a,
            maskT=maskT_a,
            ident=ident_a,
        ))

    nc = _get_nc()
    trace = os.environ.get("ALIBI_TRACE", "0") == "1"
    res = run_bass_kernel_spmd(nc, in_maps, core_ids=list(range(N_CORES)),
                               trace=trace)
    if trace:
        LAST_EXEC_NS = res.exec_time_ns

    out_full = np.empty((B, S, D), np.float32)
    for c in range(N_CORES):
        b, g = c // 2, c % 2
        out_full[b, g * (S // 2):(g + 1) * (S // 2), :] = res.results[c]["out"]
    return out_full


NPT_G = PG // 128


# revision 20
# speedup vs baseline: 1.8936x; 1.5402x over previous
"""ALiBi multi-head causal attention on 8 TRN2 NeuronCores (Bass/Tile SPMD).

Problem: B=4, S=2048, D=1024, H=16, Kd=64 (nn_AlibiMultiHeadAttention).

Sharding (identical SPMD graph on every core):
  core c -> batch b = c//2, head-group g = c%2 (heads g*8 .. g*8+7), ALL 2048 rows.
  Each core: QKV projections for its 8 heads, causal+ALiBi attention
  (transposed-logits layout: key position on PSUM partitions, query on free;
  ALiBi rides the exp() ACT bias; causal via block skipping, variable-width
  suffix matmuls, and one gpsimd affine_select per (head, s-tile) on the SBUF
  exp tile; the softmax denominator comes from 64 ones-columns in v', divided
  out with reciprocal_approx_fast), then a 2-rank AllGather within each batch
  pair exchanges attention-output halves, and each core runs the output
  projection for its half of the rows (row half = g, via partition_id-derived
  dynamic DMA offsets + per-core mine-first-permuted Wo). Host concatenates
  the 8 [1024,1024] row-slices into the full [4,2048,1024] output.

Math note: softmax is shift-invariant per row, so the ALiBi term slope*(i-j)
reduces to the key-side column term -slope*j, which is constant per PSUM
partition in the transposed-logits layout -> it rides the ACT bias operand of
exp() for free. exp() is computed without max-subtraction (logits are O(6)
for unit-normal inputs; fp32 exp overflows only at 88).
"""
import math
import os
import sys

sys.path.insert(0, "/opt/trn_rl_repo")

import numpy as np
import ml_dtypes

import concourse.bass as bass
import concourse.tile as tile
from concourse import bacc, mybir
from concourse.bass_utils import run_bass_kernel_spmd

F32 = mybir.dt.float32
BF16 = mybir.dt.bfloat16
AF = mybir.ActivationFunctionType
ALU = mybir.AluOpType

B, S, D = 4, 2048, 1024
H, KD = 16, 64
HPG = 8                 # heads per core (head-group)
PG = HPG * KD           # 512: p-slice per head-group
NT = S // 128           # 16 token tiles
ND = D // 128           # 8 d tiles
NQC = 4                 # q-chunks of 512
QC = 512
VW = 2 * KD             # v' width per head (64 + 64 ones cols -> denom on partitions 64..127)
SCALE = 1.0 / math.sqrt(KD)
NEG = -1.0e9
N_CORES = 8
GROUPS = [[0, 1], [2, 3], [4, 5], [6, 7]]
NPT_G = PG // 128

_NC = None
LAST_EXEC_NS = None


def _slopes():
    start = 2.0 ** (-(2.0 ** -(math.log2(H) - 3)))
    return np.array([start * start ** i for i in range(H)], dtype=np.float32)


def _build():
    nc = bacc.Bacc("TRN2", target_bir_lowering=False, num_devices=N_CORES)
    xq = nc.declare_dram_parameter("xq", [S, D], F32, isOutput=False)
    xk = nc.declare_dram_parameter("xk", [S, D], F32, isOutput=False)
    xv = nc.declare_dram_parameter("xv", [S, D], F32, isOutput=False)
    Wq = nc.declare_dram_parameter("Wq", [D, PG], F32, isOutput=False)
    Wk = nc.declare_dram_parameter("Wk", [D, PG], F32, isOutput=False)
    Wv = nc.declare_dram_parameter("Wv", [D, PG], F32, isOutput=False)
    Wo = nc.declare_dram_parameter("Wo", [D, D], F32, isOutput=False)
    bqr = nc.declare_dram_parameter("bqr", [128, PG // 128], F32, isOutput=False)
    bkr = nc.declare_dram_parameter("bkr", [128, PG // 128], F32, isOutput=False)
    bvr = nc.declare_dram_parameter("bvr", [128, PG], F32, isOutput=False)
    bor = nc.declare_dram_parameter("bor", [128, D], BF16, isOutput=False)
    alibi = nc.declare_dram_parameter("alibi", [128, HPG * NT], F32, isOutput=False)
    ident = nc.declare_dram_parameter("ident", [128, 128], F32, isOutput=False)
    out = nc.declare_dram_parameter("out", [S // 2, D], F32, isOutput=True)

    NPT = PG // 128  # 4 p-tiles per head-group (2 heads each)

    with tile.TileContext(nc) as tc:
        with (
            tc.tile_pool(name="const", bufs=1) as constp,
            tc.tile_pool(name="wo", bufs=1) as wop,
            tc.tile_pool(name="proj_out", bufs=1) as projp,
            tc.tile_pool(name="attn_sb", bufs=1) as attnp,
            tc.tile_pool(name="dram", bufs=1, space="DRAM") as dramp,
        ):
            alibi_t = constp.tile([128, HPG * NT], F32)
            nc.sync.dma_start(alibi_t[:], alibi[:])
            bqr_t = constp.tile([128, NPT], F32)
            nc.sync.dma_start(bqr_t[:], bqr[:])
            bkr_t = constp.tile([128, NPT], F32)
            nc.sync.dma_start(bkr_t[:], bkr[:])
            bvr_t = constp.tile([128, PG], F32)
            nc.sync.dma_start(bvr_t[:], bvr[:])
            bor_t = constp.tile([128, D], BF16)
            nc.sync.dma_start(bor_t[:], bor[:])

            # persistent projection outputs
            qT = [projp.tile([128, S], BF16, name=f"qT{p}", tag=f"qT{p}")
                  for p in range(NPT)]
            kT = [projp.tile([128, S], BF16, name=f"kT{p}", tag=f"kT{p}")
                  for p in range(NPT)]
            vp = [projp.tile([128, HPG * VW], BF16, name=f"vp{t}", tag=f"vp{t}")
                  for t in range(NT)]
            attnT = [attnp.tile([128, S], BF16, name=f"attnT{j}", tag=f"attnT{j}")
                     for j in range(NPT)]

            # ---- Phase 1: loads + DMA-transposes + projections ---------------
            # Transposes ride the HWDGE xbar (bf16 SBUF->SBUF): no PE, no PSUM,
            # no DVE. DMA order gates compute: Wk,Wq, xk, xq, Wv, xv, then Wo.
            ag_out = []
            with (
                tc.tile_pool(name="xT", bufs=2) as xtp,
                tc.tile_pool(name="wkq", bufs=1) as wkqp,
                tc.tile_pool(name="xload", bufs=3) as xlp,
                tc.tile_pool(name="ps_pj", bufs=3, space="PSUM") as ps_pj,
                tc.tile_pool(name="ps_tr", bufs=4, space="PSUM") as ps_tr,
            ):
                ident_tb = wkqp.tile([128, 128], BF16)
                nc.gpsimd.dma_start(ident_tb[:], ident[:])
                xT_all = {}
                w_tiles = {}
                def load_weight(name, W):
                    w_tiles[name] = [
                        wkqp.tile([128, PG], BF16, name=f"{name}{j}",
                                  tag=f"{name}{j}")
                        for j in range(ND)]
                    for j in range(ND):
                        wld = xlp.tile([128, PG], F32, tag="wld", name="wld", bufs=2)
                        nc.sync.dma_start(wld[:], W[j * 128:(j + 1) * 128, :])
                        nc.vector.tensor_copy(w_tiles[name][j][:], wld[:])

                load_weight("Wk", Wk)
                load_weight("Wq", Wq)

                def load_transpose(xname, x):
                    xT = [xtp.tile([128, S], BF16, name=f"{xname}T{j}",
                                   tag=f"xT{j}")
                          for j in range(ND)]
                    xT_all[xname] = xT
                    for t in range(NT):
                        xl = xlp.tile([128, D], F32, tag="xl", name="xl", bufs=2)
                        nc.sync.dma_start(xl[:], x[t * 128:(t + 1) * 128, :])
                        xlb = xlp.tile([128, D], BF16, tag="xlb", name="xlb")
                        nc.scalar.copy(xlb[:], xl[:])
                        for j in range(ND):
                            ps = ps_tr.tile([128, 128], BF16, tag="tr",
                                            name="tr")
                            nc.tensor.transpose(
                                ps[:], xlb[:, j * 128:(j + 1) * 128], ident_tb[:])
                            nc.vector.tensor_copy(
                                xT[j][:, t * 128:(t + 1) * 128], ps[:])

                def kq_proj(dstl, wname, biasr, xname):
                    for pt in range(NPT):
                        for qc in range(NQC):
                            ps = ps_pj.tile([128, QC], F32, tag="pj", name="pj")
                            for j in range(ND):
                                nc.tensor.matmul(
                                    ps[:],
                                    w_tiles[wname][j][:, pt * 128:(pt + 1) * 128],
                                    xT_all[xname][j][:, qc * QC:(qc + 1) * QC],
                                    start=(j == 0), stop=(j == ND - 1))
                            nc.vector.tensor_scalar(
                                dstl[pt][:, qc * QC:(qc + 1) * QC],
                                ps[:], biasr[:, pt:pt + 1], None, ALU.add)

                load_transpose("xk", xk)
                kq_proj(kT, "Wk", bkr_t, "xk")
                load_transpose("xq", xq)
                kq_proj(qT, "Wq", bqr_t, "xq")

                load_weight("Wv", Wv)
                load_transpose("xv", xv)
                for t in range(NT):
                    ps = ps_pj.tile([128, PG], F32, tag="pj", name="pj")
                    for j in range(ND):
                        nc.tensor.matmul(
                            ps[:], xT_all["xv"][j][:, t * 128:(t + 1) * 128],
                            w_tiles["Wv"][j][:],
                            start=(j == 0), stop=(j == ND - 1))
                    dst = vp[t][:].rearrange("a (h w) -> a h w", w=VW)[:, :, KD:VW]
                    nc.vector.tensor_tensor(
                        dst, ps[:].rearrange("a (h k) -> a h k", k=KD),
                        bvr_t[:].rearrange("a (h k) -> a h k", k=KD),
                        ALU.add)
                    ones_col = vp[t][:].rearrange(
                        "a (h w) -> a h w", w=VW)[:, :, 0:KD]
                    nc.gpsimd.memset(ones_col, 1.0)

            # ---- Phase 2: attention + per-pair AllGather ----------------------
            with (
                tc.tile_pool(name="expt", bufs=6) as expp,
                tc.tile_pool(name="attn_tmp", bufs=2) as atmp,
                tc.tile_pool(name="ps_log", bufs=2, space="PSUM") as ps_log,
                tc.tile_pool(name="ps_av", bufs=4, space="PSUM") as ps_av,
            ):
                Wo_t = [wop.tile([128, D], BF16, name=f"Wo{p}", tag=f"Wo{p}")
                        for p in range(ND)]
                for p in range(ND):
                    wol = atmp.tile([128, D], F32, tag="wol", name="wol")
                    nc.sync.dma_start(wol[:], Wo[p * 128:(p + 1) * 128, :])
                    nc.vector.tensor_copy(Wo_t[p][:], wol[:])
                pid = nc.gpsimd.partition_id()
                qoff_other = (1 - (pid & 1)) * (S // 2)
                for h in range(HPG):
                    pj = h // 2
                    off = 64 * (h % 2)
                    kT_h = kT[pj][off:off + 64, :]
                    qT_h = qT[pj][off:off + 64, :]
                    av_ps = [ps_av.tile([VW, QC], F32, tag="av", name="av")
                             for _ in range(NQC)]
                    groups = []
                    for st in range(NT):
                        qcs = list(range(st // 4, NQC))
                        for gi in range(0, len(qcs), 2):
                            groups.append((st, qcs[gi:gi + 2]))

                    def emit_av(pst, pgrp, pex):
                        prel = pst % 4
                        for k, qc in enumerate(pgrp):
                            off_k = prel * 128 if (
                                k == 0 and pgrp[0] == pst // 4) else 0
                            done = pst == 4 * qc + 3
                            nc.tensor.matmul(
                                av_ps[qc][:, off_k:QC],
                                vp[pst][:, h * VW:(h + 1) * VW],
                                pex[:, k * QC + off_k:(k + 1) * QC],
                                start=(pst == 0), stop=done)
                            if done:
                                recipb = atmp.tile([KD, QC], F32,
                                                   tag="rb", name="rb")
                                nc.vector.reciprocal_approx_fast(
                                    recipb[:], av_ps[qc][0:KD, :])
                                nc.vector.tensor_tensor(
                                    attnT[pj][off:off + 64,
                                              qc * QC:(qc + 1) * QC],
                                    av_ps[qc][KD:VW, :], recipb[:],
                                    ALU.mult)

                    pend = []  # (st, grp, expt) awaiting AV (2-deep)
                    for st, grp in groups:
                        rel = st % 4
                        diag = grp[0] == st // 4
                        start_off = rel * 128 if diag else 0
                        ps_l = ps_log.tile([128, 2 * QC], F32, tag="lg",
                                           name="lg")
                        for k, qc in enumerate(grp):
                            off_k = start_off if k == 0 else 0
                            nc.tensor.matmul(
                                ps_l[:, k * QC + off_k:(k + 1) * QC],
                                kT_h[:, st * 128:(st + 1) * 128],
                                qT_h[:, qc * QC + off_k:(qc + 1) * QC],
                                start=True, stop=True)
                        expt = expp.tile([128, 2 * QC], BF16, tag="ex",
                                         name="ex")
                        nc.scalar.activation(
                            expt[:, start_off:len(grp) * QC],
                            ps_l[:, start_off:len(grp) * QC],
                            AF.Exp,
                            bias=alibi_t[:, h * NT + st:h * NT + st + 1],
                            scale=SCALE)
                        if diag:
                            # zero upper triangle of the diagonal block
                            # (keep iff q - s >= 0) on the SBUF exp tile
                            nc.gpsimd.affine_select(
                                expt[:, start_off:start_off + 128],
                                expt[:, start_off:start_off + 128],
                                pattern=[[1, 128]],
                                compare_op=ALU.is_ge,
                                fill=0.0, base=0,
                                channel_multiplier=-1)
                        pend.append((st, grp, expt))
                        if len(pend) > 2:
                            emit_av(*pend.pop(0))
                    for pe_ in pend:
                        emit_av(*pe_)
                    if h % 2 == 1:
                        agi = dramp.tile([128, S // 2], BF16, name=f"agi{pj}",
                                         tag=f"agi{pj}")
                        ago = dramp.tile([256, S // 2], BF16, name=f"ago{pj}",
                                         tag=f"ago{pj}")
                        nc.gpsimd.dma_start(
                            agi[:], attnT[pj][:, bass.ds(qoff_other, S // 2)])
                        nc.gpsimd.collective_compute(
                            "AllGather", ALU.bypass, replica_groups=GROUPS,
                            ins=[agi.opt()], outs=[ago.opt()])
                        ag_out.append(ago)

            # ---------------- Phase 3: output projection ----------------------
            with (
                tc.tile_pool(name="attnF", bufs=1) as afp,
                tc.tile_pool(name="osb", bufs=2) as osbp,
                tc.tile_pool(name="ps_out", bufs=3, space="PSUM") as ps_out,
            ):
                pid2 = nc.gpsimd.partition_id()
                qoff_mine = (pid2 & 1) * (S // 2)
                rowoff_partner = (1 - (pid2 & 1)) * 128
                attnF = [afp.tile([128, S // 2], BF16, name=f"aF{p}", tag=f"aF{p}")
                         for p in range(ND)]
                for pj in range(NPT):
                    nc.gpsimd.dma_start(
                        attnF[pj][:], attnT[pj][:, bass.ds(qoff_mine, S // 2)])
                    nc.gpsimd.dma_start(
                        attnF[NPT + pj][:],
                        ag_out[pj][bass.ds(rowoff_partner, 128), :])
                for lt in range(ND):
                    osb = osbp.tile([128, D], F32, tag="osb", name="osb")
                    for dc in range(2):
                        ps = ps_out.tile([128, QC], F32, tag="po", name="po")
                        for p in range(ND):
                            nc.tensor.matmul(
                                ps[:], attnF[p][:, lt * 128:(lt + 1) * 128],
                                Wo_t[p][:, dc * QC:(dc + 1) * QC],
                                start=(p == 0), stop=(p == ND - 1))
                        nc.vector.tensor_tensor(
                            osb[:, dc * QC:(dc + 1) * QC], ps[:],
                            bor_t[:, dc * QC:(dc + 1) * QC], ALU.add)
                    nc.sync.dma_start(out[lt * 128:(lt + 1) * 128, :], osb[:])

    nc.finalize()
    return nc


def _get_nc():
    global _NC
    if _NC is None:
        _NC = _build()
    return _NC


def kernel(query, key, value, Wq, bq, Wk, bk, Wv, bv, Wo, bo):
    global LAST_EXEC_NS
    query = np.asarray(query, np.float32)
    key = np.asarray(key, np.float32)
    value = np.asarray(value, np.float32)
    Wq = np.asarray(Wq, np.float32)
    Wk = np.asarray(Wk, np.float32)
    Wv = np.asarray(Wv, np.float32)
    Wo = np.asarray(Wo, np.float32)
    bq = np.asarray(bq, np.float32)
    bk = np.asarray(bk, np.float32)
    bv = np.asarray(bv, np.float32)
    bo = np.asarray(bo, np.float32)

    slopes = _slopes()
    p_idx = np.arange(128, dtype=np.float32)
    bor_a = np.ascontiguousarray(np.broadcast_to(bo, (128, D)).astype(ml_dtypes.bfloat16))
    ident_a = np.eye(128, dtype=np.float32)

    in_maps = []
    for c in range(N_CORES):
        b, g = c // 2, c % 2
        sl = slice(g * PG, (g + 1) * PG)
        alibi_a = np.empty((128, HPG * NT), np.float32)
        for hl in range(HPG):
            sp = slopes[g * HPG + hl]
            for st in range(NT):
                alibi_a[:, hl * NT + st] = -sp * (st * 128 + p_idx)
        in_maps.append(dict(
            xq=np.ascontiguousarray(query[b]),
            xk=np.ascontiguousarray(key[b]),
            xv=np.ascontiguousarray(value[b]),
            Wq=np.ascontiguousarray(Wq[:, sl]),
            Wk=np.ascontiguousarray(Wk[:, sl]),
            Wv=np.ascontiguousarray(Wv[:, sl]),
            Wo=np.ascontiguousarray(
                np.concatenate([Wo[g * PG:(g + 1) * PG, :],
                                Wo[(1 - g) * PG:(2 - g) * PG, :]], axis=0)),
            bqr=np.ascontiguousarray(bq[sl].reshape(NPT_G, 128).T),
            bkr=np.ascontiguousarray(bk[sl].reshape(NPT_G, 128).T),
            bvr=np.ascontiguousarray(np.broadcast_to(bv[sl], (128, PG))),
            bor=bor_a,
            alibi=alibi_a,
            ident=ident_a,
        ))

    nc = _get_nc()
    trace = os.environ.get("ALIBI_TRACE", "0") == "1"
    res = run_bass_kernel_spmd(nc, in_maps, core_ids=list(range(N_CORES)),
                               trace=trace)
    if trace:
        LAST_EXEC_NS = res.exec_time_ns

    out_full = np.empty((B, S, D), np.float32)
    for c in range(N_CORES):
        b, g = c // 2, c % 2
        out_full[b, g * (S // 2):(g + 1) * (S // 2), :] = res.results[c]["out"]
    return out_full


NPT_G = PG // 128
